# revision 20
# baseline (speedup 1.0000x reference)
"""Trainium2 Bass kernel for nn_BatchFlipLoss (NCE batch-flip loss + CE loss).

Math reformulation (validated ~1e-7 vs the jax reference in f64; the
first-order series below adds ~9e-5, vs a 2e-2 gate):

The reference sums BatchCriterion over 36 flip-class pairs (i,j), j>=i.
For pair (i,j) with x = [f_i; f_j] (f_c = features[c::8], L2-normalized,
B=512 rows each), T=0.1, the loss decomposes over ordered halves (a,b).
With E_ab = exp(10*G_ab), G_ab = f_a@f_b.T, S_ab = rowsum(E_ab),
d_ab[p] = f_a[p].f_b[p]:

  D_ab = S0_aa + S_ab      (S0_aa: diag-removed; (a,a): D = 2*S0_aa+e^10)
  half = 10*d - ln(D) - N1/D - ln(1 - exp(10 d)/D),  N1 = S0_aa + S_ab
  (a,a) pair = 2*(10*d - lnD - 2*S0_aa/D)
  ln(1-x) ~ -x only (the x^2/2 tail is ~9e-5 relative after scaling).

Work assignment: 36 unordered blocks over 8 cores = 4.5 each. Core c
computes blocks (c, c+j) j=0..3 in full (16 [128,512] Gram tiles) plus
HALF of its distance-4 block {c, c+4}: cores 0-3 take columns 0:256 of
E(f_c rows x f_{c+4} cols), cores 4-7 take rows 256:512 of the mirror
E(f_c rows x f_{c-4} cols) — identical instruction stream, different
host-packed lhsT/rhs inputs (four [128,256] matmuls each).

Device pipeline per core (SPMD, inputs rotated so own class is block 0):
  - Gram matmuls write 1-3 tiles into multi-bank PSUM tiles; ONE ACT exp
    per group ([128,512..1536]) converts to bf16 E in SBUF.
  - per-tile rowsums: DVE tensor_scalar(+accum_out) on bf16 E (4x DVE
    mode, accum free) -> out[:, t].
  - colsums for j in {1,2,3} (the partner core's rowsums): PE matmuls
    with one-hot lhsT accumulate into one [3,512] PSUM tile; the d4
    half-block colsums accumulate into a separate [2,256] PSUM tile.
  - CE: one ACT exp over [128,1600] predicts + DVE accum per 400-chunk.
  - diag of block (c,c) is NOT zeroed on device: the host subtracts
    bf16(exp(10*||f_p||^2_bf16)) from the raw diag rowsums.
The host does only O(N*D)/O(N) work: input layout, d_ab products, the
CE label gather, and the closed-form scalar combine.
"""

from contextlib import ExitStack

import numpy as np

FLIP = 8
B = 512
D = 128
C = 400
N = 4096
ALPHA = 0.03
E10 = float(np.exp(np.float32(10.0)))

# ftp column layout (bf16): 4 full blocks, d4 lhsT chunks, d4 rhs halves,
# one-hot columns for the j-colsums (3x3) and d4-colsums (2x2).
BLK = 4 * B            # 0:2048   blocks j=0..3
LHS4 = BLK             # 2048:2560
RHS4 = LHS4 + B        # 2560:3072
OHJ = RHS4 + B         # 3072:3081
OH4 = OHJ + 9          # 3081:3085
FT_COLS = 3088

# slot ids: s0=(j0,r0) | s1..s12 = j1r0..j3r3 | s13,s14 = d4 halves |
# s15..s17 = (j0,r1..r3).  outt col t for slot rowsums (d4 slots use two
# cols each: 13,14 and 15,16), CE sums in cols 20:24.
# Group order is tuned for the DMA arrival schedule and the two pipeline
# tails: early groups need only feature block 0/1, the d4 slots wait for
# the third ft DMA, the j-colsum chain ends one group before the last so
# its staging+DMA overlaps the final exp, and the last group is a diag
# tile (no colsums) so only its rowsum gates the final output DMA.
GROUPS = [[0], [15, 16, 1], [2, 3, 4], [13, 14, 5], [6, 7, 8],
          [9, 10, 11], [12], [17]]
J0_COL = {0: 0, 1: 17, 2: 18, 3: 19}

_CACHE = {}


def _build_nc():
    import concourse.tile as tile
    from concourse import bacc, mybir

    f32 = mybir.dt.float32
    bf16 = mybir.dt.bfloat16
    AF = mybir.ActivationFunctionType
    OP = mybir.AluOpType

    nc = bacc.Bacc("TRN2", target_bir_lowering=False, debug=False)

    ft_d = nc.dram_tensor("ft", [D, FT_COLS], bf16, kind="ExternalInput")
    pred_d = nc.dram_tensor("pred", [128, 4 * C], f32, kind="ExternalInput")
    out_d = nc.dram_tensor("out", [128, 24], f32, kind="ExternalOutput")
    cs1_d = nc.dram_tensor("cs1", [5, B], f32, kind="ExternalOutput")

    def slot_info(s):
        """-> (kind, j, r) with kind in {'full','d4'}"""
        if s == 0:
            return ("full", 0, 0)
        if 1 <= s <= 12:
            return ("full", 1 + (s - 1) // 4, (s - 1) % 4)
        if s in (13, 14):
            return ("d4", None, s - 13)
        return ("full", 0, s - 14)

    with tile.TileContext(nc) as tc, ExitStack() as ctx:
        const = ctx.enter_context(tc.tile_pool(name="const", bufs=1))
        gpool = ctx.enter_context(tc.tile_pool(name="gp", bufs=2, space="PSUM"))
        cpool = ctx.enter_context(tc.tile_pool(name="cp", bufs=1, space="PSUM"))
        c4pool = ctx.enter_context(tc.tile_pool(name="c4", bufs=1, space="PSUM"))
        epool = ctx.enter_context(tc.tile_pool(name="ep", bufs=4))
        small = ctx.enter_context(tc.tile_pool(name="sm", bufs=1))

        ftt = const.tile([D, FT_COLS], bf16)
        predt = const.tile([128, 4 * C], f32)
        outt = small.tile([128, 24], f32)
        scr = small.tile([128, 3 * B], bf16)
        scrp = small.tile([128, C], bf16)
        cs1s = small.tile([3, B], f32)
        cs4s = small.tile([2, 256], f32)
        nc.sync.dma_start(ftt[:, 0:B], ft_d[:, 0:B])
        nc.sync.dma_start(ftt[:, B : 2 * B], ft_d[:, B : 2 * B])
        nc.sync.dma_start(ftt[:, 2 * B : FT_COLS], ft_d[:, 2 * B : FT_COLS])
        nc.sync.dma_start(predt[:], pred_d[:, :])

        cs1p = cpool.tile([3, B], f32)
        cs4p = c4pool.tile([2, 256], f32)

        pg = {}
        eg = {}

        def emit_mms(g):
            slots = GROUPS[g]
            pgt = gpool.tile([128, len(slots) * B], f32, tag="pg")
            pg[g] = pgt
            for i, s in enumerate(slots):
                kind, j, r = slot_info(s)
                if kind == "full":
                    nc.tensor.matmul(
                        pgt[:, i * B : (i + 1) * B],
                        ftt[:, r * 128 : (r + 1) * 128],
                        ftt[:, j * B : (j + 1) * B],
                        start=True,
                        stop=True,
                    )
                else:  # d4 half-slot: two [128,256] matmuls (quarters 2r,2r+1)
                    for h in range(2):
                        q = 2 * r + h
                        nc.tensor.matmul(
                            pgt[:, i * B + h * 256 : i * B + (h + 1) * 256],
                            ftt[:, LHS4 + q * 128 : LHS4 + (q + 1) * 128],
                            ftt[:, RHS4 + (q // 2) * 256 : RHS4 + (q // 2 + 1) * 256],
                            start=True,
                            stop=True,
                        )

        def emit_exp(g):
            egt = epool.tile([128, len(GROUPS[g]) * B], bf16, tag="eg")
            eg[g] = egt
            nc.scalar.activation(egt[:], pg[g][:], AF.Exp, bias=0.0, scale=10.0)

        def emit_rs(g):
            for i, s in enumerate(GROUPS[g]):
                kind, j, r = slot_info(s)
                if kind == "full":
                    col = J0_COL[r] if j == 0 else s
                    nc.vector.tensor_scalar(
                        scr[:, i * B : (i + 1) * B],
                        eg[g][:, i * B : (i + 1) * B],
                        1.0, None, OP.mult, OP.add,
                        accum_out=outt[:, col : col + 1],
                    )
                else:
                    for h in range(2):
                        col = 13 + 2 * r + h
                        nc.vector.tensor_scalar(
                            scr[:, i * B + h * 256 : i * B + (h + 1) * 256],
                            eg[g][:, i * B + h * 256 : i * B + (h + 1) * 256],
                            1.0, None, OP.mult, OP.add,
                            accum_out=outt[:, col : col + 1],
                        )

        def emit_cs(g):
            for i, s in enumerate(GROUPS[g]):
                kind, j, r = slot_info(s)
                if kind == "full":
                    if not (1 <= j <= 3):
                        continue
                    oh = OHJ + 3 * (j - 1)
                    nc.tensor.matmul(
                        cs1p[:],
                        ftt[:, oh : oh + 3],
                        eg[g][:, i * B : (i + 1) * B],
                        start=(s == 1),
                        stop=(s == 12),
                    )
                else:
                    for h in range(2):
                        q = 2 * r + h
                        oh = OH4 + 2 * (q // 2)
                        nc.tensor.matmul(
                            cs4p[:],
                            ftt[:, oh : oh + 2],
                            eg[g][:, i * B + h * 256 : i * B + (h + 1) * 256],
                            start=(q == 0),
                            stop=(q == 3),
                        )

        emit_mms(0)
        emit_mms(1)
        emit_exp(0)
        emit_rs(0)
        for g in range(2, len(GROUPS)):
            emit_mms(g)
            emit_exp(g - 1)
            emit_rs(g - 1)
            if g >= 3:
                # colsum matmuls one group late so they never sit ahead of
                # the next group's Gram matmuls in PE's in-order stream
                emit_cs(g - 2)
            if g == 4:
                # CE: predicts arrive behind ft; slot the exp mid-pipeline.
                ept = epool.tile([128, 4 * C], bf16, tag="ept")
                nc.scalar.activation(ept[:], predt[:], AF.Exp, bias=0.0, scale=1.0)
                for cchunk in range(4):
                    nc.vector.tensor_scalar(
                        scrp[:],
                        ept[:, cchunk * C : (cchunk + 1) * C],
                        1.0, None, OP.mult, OP.add,
                        accum_out=outt[:, 20 + cchunk : 21 + cchunk],
                    )
            if g == 7:
                # d4 colsums completed back in group 3; stage + DMA early
                nc.vector.tensor_copy(cs4s[:], cs4p[:])
                nc.gpsimd.dma_start(cs1_d[3:5, 0:256], cs4s[:])
        gl = len(GROUPS) - 1
        emit_exp(gl)
        emit_rs(gl)
        emit_cs(gl - 1)  # the j3r3 stop matmul (group G)
        # stage the j-colsums on ACT (free after the last exp); DVE is
        # reserved for the final rowsum that gates the output DMA
        nc.scalar.copy(cs1s[:], cs1p[:])
        nc.gpsimd.dma_start(cs1_d[0:3, :], cs1s[:])

        nc.sync.dma_start(out_d[:, :], outt[:])

    nc.compile()
    return nc


def _get_nc():
    if "nc" not in _CACHE:
        _CACHE["nc"] = _build_nc()
    return _CACHE["nc"]


def _prep_in_maps(predicts, labels, features):
    import ml_dtypes

    feats = np.ascontiguousarray(features, dtype=np.float32)
    pred = np.ascontiguousarray(predicts, dtype=np.float32)
    f8 = feats.reshape(B, FLIP, D).transpose(1, 0, 2)  # [8,512,128], f8[c]=feats[c::8]

    ohj = np.zeros((D, 9), dtype=np.float32)
    for j in (1, 2, 3):
        ohj[:, 3 * (j - 1) + (j - 1)] = 1.0
    oh4 = np.zeros((D, 4), dtype=np.float32)
    oh4[:, 0] = 1.0  # quarters 0,1 -> cs4 row 0
    oh4[:, 3] = 1.0  # quarters 2,3 -> cs4 row 1

    in_maps = []
    for a in range(FLIP):
        order = [(a + i) % FLIP for i in range(4)]
        fo = f8[order]                       # [4, 512, 128] blocks j=0..3
        fp = f8[(a + 4) % FLIP]              # d4 partner [512, 128]
        ft = np.zeros((D, FT_COLS), dtype=np.float32)
        ft[:, :BLK] = fo.transpose(2, 0, 1).reshape(D, BLK)
        if a < 4:
            # columns 0:256 of E(f_a rows x f_partner cols): lhsT chunks =
            # own rows 0..3, rhs halves both = partner[0:256]
            ft[:, LHS4:RHS4] = f8[a].T
            ft[:, RHS4 : RHS4 + 256] = fp[0:256].T
            ft[:, RHS4 + 256 : OHJ] = fp[0:256].T
        else:
            # rows 256:512 of E(f_a rows x f_partner cols), all 512 columns
            own = f8[a]
            ft[:, LHS4 : LHS4 + 128] = own[256:384].T
            ft[:, LHS4 + 128 : LHS4 + 256] = own[384:512].T
            ft[:, LHS4 + 256 : LHS4 + 384] = own[256:384].T
            ft[:, LHS4 + 384 : RHS4] = own[384:512].T
            ft[:, RHS4:OHJ] = fp.T
        ft[:, OHJ:OH4] = ohj
        ft[:, OH4 : OH4 + 4] = oh4
        pb = pred[a * B : (a + 1) * B].reshape(4, 128, C).transpose(1, 0, 2)
        in_maps.append(
            {
                "ft": np.ascontiguousarray(ft).astype(ml_dtypes.bfloat16),
                "pred": np.ascontiguousarray(pb.reshape(128, 4 * C)),
            }
        )
    return in_maps


def _combine(outs, predicts, labels, features):
    """Host-side O(N*D) combine: reroute per-block sums between the
    ordered halves and apply the closed-form first-order series."""
    import ml_dtypes

    feats = np.asarray(features, dtype=np.float32)
    f8 = feats.reshape(B, FLIP, D).transpose(1, 0, 2).astype(np.float64)
    fb8 = f8.astype(ml_dtypes.bfloat16).astype(np.float64)  # device-side values

    dv = np.einsum("apd,bpd->abp", f8, f8)

    S1 = {}
    m = {}
    cs = {}
    for c in range(FLIP):
        m[c] = np.asarray(outs[c]["out"], np.float64)   # [128, 24]
        cs[c] = np.asarray(outs[c]["cs1"], np.float64)  # [5, 512]
        for j in range(4):
            b = (c + j) % FLIP
            if j == 0:
                cols = [J0_COL[r] for r in range(4)]
            else:
                cols = [1 + 4 * (j - 1) + r for r in range(4)]
            S1[(c, b)] = m[c][:, cols].T.reshape(B)
        for j in (1, 2, 3):
            S1[((c + j) % FLIP, c)] = cs[c][j - 1]

    # distance-4 pairs {b, b+4}, b < 4: stitch the two half-blocks
    for b in range(4):
        bp = b + 4
        # rowsums of M = E(f_b rows x f_bp cols)
        partial = m[b][:, 13:17].T.reshape(B)          # cols 0:256, all rows
        compl_lo = cs[bp][3, 0:256]                    # rows' missing cols, q<256
        compl_hi = cs[bp][4, 0:256]                    # q in 256:512
        S1[(b, bp)] = partial + np.concatenate([compl_lo, compl_hi])
        # rowsums of M^T = E(f_bp rows x f_b cols)
        lo = cs[b][3, 0:256] + cs[b][4, 0:256]         # rows 0:256 of f_bp
        hi = np.empty(256)
        hi[0:128] = m[bp][:, 13] + m[bp][:, 15]        # rows 256:384
        hi[128:256] = m[bp][:, 14] + m[bp][:, 16]      # rows 384:512
        S1[(bp, b)] = np.concatenate([lo, hi])

    # remove the raw diagonal exp from the own-block rowsums the same way
    # the device accumulated it: bf16(exp(10*||f_p||^2 in bf16 products))
    S10 = {}
    for c in range(FLIP):
        gpp = np.einsum("pd,pd->p", fb8[c], fb8[c])
        dg = np.exp(10.0 * gpp).astype(np.float32)
        dg = dg.astype(ml_dtypes.bfloat16).astype(np.float64)
        S10[c] = S1[(c, c)] - dg

    nce = 0.0
    for a in range(FLIP):
        for b in range(FLIP):
            d = dv[a, b]
            if a == b:
                N1 = 2.0 * S10[a]
                Dv = N1 + E10
                half = 10.0 * d - np.log(Dv) - N1 / Dv
                nce += 2.0 * half.sum()
            else:
                N1 = S10[a] + S1[(a, b)]
                half = (
                    10.0 * d
                    - np.log(N1)
                    - 1.0
                    - np.log1p(-np.exp(10.0 * d) / N1)
                )
                nce += half.sum()

    # CE: device exp-sums + host label gather
    pred = np.asarray(predicts, dtype=np.float64)
    lab = np.asarray(labels).astype(np.int64)
    xl = pred[np.arange(N), lab]
    ce = -xl.sum()
    for c in range(FLIP):
        se = m[c][:, 20:24]  # se[p, cc] = sum_k exp(pred[c*512+cc*128+p, k])
        ce += np.log(se).T.reshape(B).sum()

    val = ALPHA * (-(nce) / 1024.0) + ce / N
    return np.array(val, dtype=np.float32)


def _run_hw(in_maps, trace=False):
    from concourse.bass_utils import run_bass_kernel_spmd

    nc = _get_nc()
    return run_bass_kernel_spmd(nc, in_maps, core_ids=list(range(FLIP)), trace=trace)


def kernel(predicts, labels, features, indexs=None, **_):
    in_maps = _prep_in_maps(predicts, labels, features)
    res = _run_hw(in_maps)
    return _combine(res.results, predicts, labels, features)


def kernel_sim(predicts, labels, features, indexs=None, **_):
    """CoreSim (CPU simulator) path for fast correctness iteration."""
    from concourse.bass_interp import CoreSim

    nc = _get_nc()
    in_maps = _prep_in_maps(predicts, labels, features)
    outs = []
    for a in range(FLIP):
        sim = CoreSim(nc, trace=False)
        for k, v in in_maps[a].items():
            sim.tensor(k)[:] = v
        sim.simulate()
        outs.append({k: np.array(sim.tensor(k)) for k in ("out", "cs1")})
    return _combine(outs, predicts, labels, features)


# revision 22
# speedup vs baseline: 1.0007x; 1.0007x over previous
"""Trainium2 Bass kernel for nn_BatchFlipLoss (NCE batch-flip loss + CE loss).

Math reformulation (validated ~1e-7 vs the jax reference in f64; the
first-order series below adds ~9e-5, vs a 2e-2 gate):

The reference sums BatchCriterion over 36 flip-class pairs (i,j), j>=i.
For pair (i,j) with x = [f_i; f_j] (f_c = features[c::8], L2-normalized,
B=512 rows each), T=0.1, the loss decomposes over ordered halves (a,b).
With E_ab = exp(10*G_ab), G_ab = f_a@f_b.T, S_ab = rowsum(E_ab),
d_ab[p] = f_a[p].f_b[p]:

  D_ab = S0_aa + S_ab      (S0_aa: diag-removed; (a,a): D = 2*S0_aa+e^10)
  half = 10*d - ln(D) - N1/D - ln(1 - exp(10 d)/D),  N1 = S0_aa + S_ab
  (a,a) pair = 2*(10*d - lnD - 2*S0_aa/D)
  ln(1-x) ~ -x only (the x^2/2 tail is ~9e-5 relative after scaling).

Work assignment: 36 unordered blocks over 8 cores = 4.5 each. Core c
computes blocks (c, c+j) j=0..3 in full (16 [128,512] Gram tiles) plus
HALF of its distance-4 block {c, c+4}: cores 0-3 take columns 0:256 of
E(f_c rows x f_{c+4} cols), cores 4-7 take rows 256:512 of the mirror
E(f_c rows x f_{c-4} cols) — identical instruction stream, different
host-packed lhsT/rhs inputs (four [128,256] matmuls each).

Device pipeline per core (SPMD, inputs rotated so own class is block 0):
  - Gram matmuls write 1-3 tiles into multi-bank PSUM tiles; ONE ACT exp
    per group ([128,512..1536]) converts to bf16 E in SBUF.
  - per-tile rowsums: DVE tensor_scalar(+accum_out) on bf16 E (4x DVE
    mode, accum free) -> out[:, t].
  - colsums for j in {1,2,3} (the partner core's rowsums): PE matmuls
    with one-hot lhsT accumulate into one [3,512] PSUM tile; the d4
    half-block colsums accumulate into a separate [2,256] PSUM tile.
  - CE: one ACT exp over [128,1600] predicts + DVE accum per 400-chunk.
  - diag of block (c,c) is NOT zeroed on device: the host subtracts
    bf16(exp(10*||f_p||^2_bf16)) from the raw diag rowsums.
The host does only O(N*D)/O(N) work: input layout, d_ab products, the
CE label gather, and the closed-form scalar combine.
"""

from contextlib import ExitStack

import numpy as np

FLIP = 8
B = 512
D = 128
C = 400
N = 4096
ALPHA = 0.03
E10 = float(np.exp(np.float32(10.0)))

# ftp column layout (bf16): 4 full blocks, d4 lhsT chunks, d4 rhs halves,
# one-hot columns for the j-colsums (3x3) and d4-colsums (2x2).
BLK = 4 * B            # 0:2048   blocks j=0..3
LHS4 = BLK             # 2048:2560
RHS4 = LHS4 + B        # 2560:3072
OHJ = RHS4 + B         # 3072:3081
OH4 = OHJ + 9          # 3081:3085
FT_COLS = 3088

# slot ids: s0=(j0,r0) | s1..s12 = j1r0..j3r3 | s13,s14 = d4 halves |
# s15..s17 = (j0,r1..r3).  outt col t for slot rowsums (d4 slots use two
# cols each: 13,14 and 15,16), CE sums in cols 20:24.
# Group order is tuned for the DMA arrival schedule and the two pipeline
# tails: early groups need only feature block 0/1, the d4 slots wait for
# the third ft DMA, the j-colsum chain ends one group before the last so
# its staging+DMA overlaps the final exp, and the last group is a diag
# tile (no colsums) so only its rowsum gates the final output DMA.
GROUPS = [[0], [15, 16, 1], [2, 3, 4], [13, 14, 5], [6, 7, 8],
          [9, 10, 11], [12], [17]]
J0_COL = {0: 0, 1: 17, 2: 18, 3: 19}

_CACHE = {}


def _build_nc():
    import concourse.tile as tile
    from concourse import bacc, mybir

    f32 = mybir.dt.float32
    bf16 = mybir.dt.bfloat16
    AF = mybir.ActivationFunctionType
    OP = mybir.AluOpType

    nc = bacc.Bacc("TRN2", target_bir_lowering=False, debug=False)

    ft_d = nc.dram_tensor("ft", [D, FT_COLS], bf16, kind="ExternalInput")
    pred_d = nc.dram_tensor("pred", [128, 4 * C], f32, kind="ExternalInput")
    out_d = nc.dram_tensor("out", [128, 24], f32, kind="ExternalOutput")
    cs1_d = nc.dram_tensor("cs1", [5, B], f32, kind="ExternalOutput")

    def slot_info(s):
        """-> (kind, j, r) with kind in {'full','d4'}"""
        if s == 0:
            return ("full", 0, 0)
        if 1 <= s <= 12:
            return ("full", 1 + (s - 1) // 4, (s - 1) % 4)
        if s in (13, 14):
            return ("d4", None, s - 13)
        return ("full", 0, s - 14)

    with tile.TileContext(nc) as tc, ExitStack() as ctx:
        const = ctx.enter_context(tc.tile_pool(name="const", bufs=1))
        gpool = ctx.enter_context(tc.tile_pool(name="gp", bufs=2, space="PSUM"))
        cpool = ctx.enter_context(tc.tile_pool(name="cp", bufs=1, space="PSUM"))
        c4pool = ctx.enter_context(tc.tile_pool(name="c4", bufs=1, space="PSUM"))
        epool = ctx.enter_context(tc.tile_pool(name="ep", bufs=4))
        small = ctx.enter_context(tc.tile_pool(name="sm", bufs=1))

        ftt = const.tile([D, FT_COLS], bf16)
        predt = const.tile([128, 4 * C], f32)
        outt = small.tile([128, 24], f32)
        scr = small.tile([128, 3 * B], bf16)
        scrp = small.tile([128, C], bf16)
        cs1s = small.tile([3, B], f32)
        cs4s = small.tile([2, 256], f32)
        nc.sync.dma_start(ftt[:, 0:B], ft_d[:, 0:B])
        nc.sync.dma_start(ftt[:, B : 2 * B], ft_d[:, B : 2 * B])
        nc.sync.dma_start(ftt[:, 2 * B : FT_COLS], ft_d[:, 2 * B : FT_COLS])
        nc.sync.dma_start(predt[:], pred_d[:, :])

        cs1p = cpool.tile([3, B], f32)
        cs4p = c4pool.tile([2, 256], f32)

        pg = {}
        eg = {}

        def emit_mms(g):
            slots = GROUPS[g]
            pgt = gpool.tile([128, len(slots) * B], f32, tag="pg")
            pg[g] = pgt
            for i, s in enumerate(slots):
                kind, j, r = slot_info(s)
                if kind == "full":
                    nc.tensor.matmul(
                        pgt[:, i * B : (i + 1) * B],
                        ftt[:, r * 128 : (r + 1) * 128],
                        ftt[:, j * B : (j + 1) * B],
                        start=True,
                        stop=True,
                    )
                else:  # d4 half-slot: two [128,256] matmuls (quarters 2r,2r+1)
                    for h in range(2):
                        q = 2 * r + h
                        nc.tensor.matmul(
                            pgt[:, i * B + h * 256 : i * B + (h + 1) * 256],
                            ftt[:, LHS4 + q * 128 : LHS4 + (q + 1) * 128],
                            ftt[:, RHS4 + (q // 2) * 256 : RHS4 + (q // 2 + 1) * 256],
                            start=True,
                            stop=True,
                        )

        def emit_exp(g, accum=None):
            egt = epool.tile([128, len(GROUPS[g]) * B], bf16, tag="eg")
            eg[g] = egt
            nc.scalar.activation(
                egt[:], pg[g][:], AF.Exp, bias=0.0, scale=10.0, accum_out=accum
            )

        def emit_rs(g):
            for i, s in enumerate(GROUPS[g]):
                kind, j, r = slot_info(s)
                if kind == "full":
                    col = J0_COL[r] if j == 0 else s
                    nc.vector.tensor_scalar(
                        scr[:, i * B : (i + 1) * B],
                        eg[g][:, i * B : (i + 1) * B],
                        1.0, None, OP.mult, OP.add,
                        accum_out=outt[:, col : col + 1],
                    )
                else:
                    for h in range(2):
                        col = 13 + 2 * r + h
                        nc.vector.tensor_scalar(
                            scr[:, i * B + h * 256 : i * B + (h + 1) * 256],
                            eg[g][:, i * B + h * 256 : i * B + (h + 1) * 256],
                            1.0, None, OP.mult, OP.add,
                            accum_out=outt[:, col : col + 1],
                        )

        def emit_cs(g):
            for i, s in enumerate(GROUPS[g]):
                kind, j, r = slot_info(s)
                if kind == "full":
                    if not (1 <= j <= 3):
                        continue
                    oh = OHJ + 3 * (j - 1)
                    nc.tensor.matmul(
                        cs1p[:],
                        ftt[:, oh : oh + 3],
                        eg[g][:, i * B : (i + 1) * B],
                        start=(s == 1),
                        stop=(s == 12),
                    )
                else:
                    for h in range(2):
                        q = 2 * r + h
                        oh = OH4 + 2 * (q // 2)
                        nc.tensor.matmul(
                            cs4p[:],
                            ftt[:, oh : oh + 2],
                            eg[g][:, i * B + h * 256 : i * B + (h + 1) * 256],
                            start=(q == 0),
                            stop=(q == 3),
                        )

        emit_mms(0)
        emit_mms(1)
        emit_exp(0)
        emit_rs(0)
        for g in range(2, len(GROUPS)):
            emit_mms(g)
            emit_exp(g - 1)
            emit_rs(g - 1)
            if g >= 3:
                # colsum matmuls one group late so they never sit ahead of
                # the next group's Gram matmuls in PE's in-order stream
                emit_cs(g - 2)
            if g == 4:
                # CE: predicts arrive behind ft; slot the exp mid-pipeline.
                ept = epool.tile([128, 4 * C], bf16, tag="ept")
                nc.scalar.activation(ept[:], predt[:], AF.Exp, bias=0.0, scale=1.0)
                for cchunk in range(4):
                    nc.vector.tensor_scalar(
                        scrp[:],
                        ept[:, cchunk * C : (cchunk + 1) * C],
                        1.0, None, OP.mult, OP.add,
                        accum_out=outt[:, 20 + cchunk : 21 + cchunk],
                    )
            if g == 7:
                # d4 colsums completed back in group 3; stage + DMA early
                nc.vector.tensor_copy(cs4s[:], cs4p[:])
                nc.gpsimd.dma_start(cs1_d[3:5, 0:256], cs4s[:])
        emit_cs(6)  # the j3r3 stop matmul (group G)
        nc.vector.tensor_copy(cs1s[:], cs1p[:])
        nc.gpsimd.dma_start(cs1_d[0:3, :], cs1s[:])
        # last group is a single diag tile: fold its rowsum into the exp's
        # own accumulator so only ACT gates the output DMA
        gl = len(GROUPS) - 1
        emit_exp(gl, accum=outt[:, 19:20])

        nc.sync.dma_start(out_d[:, :], outt[:])

    nc.compile()
    return nc


def _get_nc():
    if "nc" not in _CACHE:
        _CACHE["nc"] = _build_nc()
    return _CACHE["nc"]


def _prep_in_maps(predicts, labels, features):
    import ml_dtypes

    feats = np.ascontiguousarray(features, dtype=np.float32)
    pred = np.ascontiguousarray(predicts, dtype=np.float32)
    f8 = feats.reshape(B, FLIP, D).transpose(1, 0, 2)  # [8,512,128], f8[c]=feats[c::8]

    ohj = np.zeros((D, 9), dtype=np.float32)
    for j in (1, 2, 3):
        ohj[:, 3 * (j - 1) + (j - 1)] = 1.0
    oh4 = np.zeros((D, 4), dtype=np.float32)
    oh4[:, 0] = 1.0  # quarters 0,1 -> cs4 row 0
    oh4[:, 3] = 1.0  # quarters 2,3 -> cs4 row 1

    in_maps = []
    for a in range(FLIP):
        order = [(a + i) % FLIP for i in range(4)]
        fo = f8[order]                       # [4, 512, 128] blocks j=0..3
        fp = f8[(a + 4) % FLIP]              # d4 partner [512, 128]
        ft = np.zeros((D, FT_COLS), dtype=np.float32)
        ft[:, :BLK] = fo.transpose(2, 0, 1).reshape(D, BLK)
        if a < 4:
            # columns 0:256 of E(f_a rows x f_partner cols): lhsT chunks =
            # own rows 0..3, rhs halves both = partner[0:256]
            ft[:, LHS4:RHS4] = f8[a].T
            ft[:, RHS4 : RHS4 + 256] = fp[0:256].T
            ft[:, RHS4 + 256 : OHJ] = fp[0:256].T
        else:
            # rows 256:512 of E(f_a rows x f_partner cols), all 512 columns
            own = f8[a]
            ft[:, LHS4 : LHS4 + 128] = own[256:384].T
            ft[:, LHS4 + 128 : LHS4 + 256] = own[384:512].T
            ft[:, LHS4 + 256 : LHS4 + 384] = own[256:384].T
            ft[:, LHS4 + 384 : RHS4] = own[384:512].T
            ft[:, RHS4:OHJ] = fp.T
        ft[:, OHJ:OH4] = ohj
        ft[:, OH4 : OH4 + 4] = oh4
        pb = pred[a * B : (a + 1) * B].reshape(4, 128, C).transpose(1, 0, 2)
        in_maps.append(
            {
                "ft": np.ascontiguousarray(ft).astype(ml_dtypes.bfloat16),
                "pred": np.ascontiguousarray(pb.reshape(128, 4 * C)),
            }
        )
    return in_maps


def _combine(outs, predicts, labels, features):
    """Host-side O(N*D) combine: reroute per-block sums between the
    ordered halves and apply the closed-form first-order series."""
    import ml_dtypes

    feats = np.asarray(features, dtype=np.float32)
    f8 = feats.reshape(B, FLIP, D).transpose(1, 0, 2).astype(np.float64)
    fb8 = f8.astype(ml_dtypes.bfloat16).astype(np.float64)  # device-side values

    dv = np.einsum("apd,bpd->abp", f8, f8)

    S1 = {}
    m = {}
    cs = {}
    for c in range(FLIP):
        m[c] = np.asarray(outs[c]["out"], np.float64)   # [128, 24]
        cs[c] = np.asarray(outs[c]["cs1"], np.float64)  # [5, 512]
        for j in range(4):
            b = (c + j) % FLIP
            if j == 0:
                cols = [J0_COL[r] for r in range(4)]
            else:
                cols = [1 + 4 * (j - 1) + r for r in range(4)]
            S1[(c, b)] = m[c][:, cols].T.reshape(B)
        for j in (1, 2, 3):
            S1[((c + j) % FLIP, c)] = cs[c][j - 1]

    # distance-4 pairs {b, b+4}, b < 4: stitch the two half-blocks
    for b in range(4):
        bp = b + 4
        # rowsums of M = E(f_b rows x f_bp cols)
        partial = m[b][:, 13:17].T.reshape(B)          # cols 0:256, all rows
        compl_lo = cs[bp][3, 0:256]                    # rows' missing cols, q<256
        compl_hi = cs[bp][4, 0:256]                    # q in 256:512
        S1[(b, bp)] = partial + np.concatenate([compl_lo, compl_hi])
        # rowsums of M^T = E(f_bp rows x f_b cols)
        lo = cs[b][3, 0:256] + cs[b][4, 0:256]         # rows 0:256 of f_bp
        hi = np.empty(256)
        hi[0:128] = m[bp][:, 13] + m[bp][:, 15]        # rows 256:384
        hi[128:256] = m[bp][:, 14] + m[bp][:, 16]      # rows 384:512
        S1[(bp, b)] = np.concatenate([lo, hi])

    # remove the raw diagonal exp from the own-block rowsums the same way
    # the device accumulated it: bf16(exp(10*||f_p||^2 in bf16 products))
    S10 = {}
    for c in range(FLIP):
        gpp = np.einsum("pd,pd->p", fb8[c], fb8[c])
        dg = np.exp(10.0 * gpp).astype(np.float32)
        dg = dg.astype(ml_dtypes.bfloat16).astype(np.float64)
        S10[c] = S1[(c, c)] - dg

    nce = 0.0
    for a in range(FLIP):
        for b in range(FLIP):
            d = dv[a, b]
            if a == b:
                N1 = 2.0 * S10[a]
                Dv = N1 + E10
                half = 10.0 * d - np.log(Dv) - N1 / Dv
                nce += 2.0 * half.sum()
            else:
                N1 = S10[a] + S1[(a, b)]
                half = (
                    10.0 * d
                    - np.log(N1)
                    - 1.0
                    - np.log1p(-np.exp(10.0 * d) / N1)
                )
                nce += half.sum()

    # CE: device exp-sums + host label gather
    pred = np.asarray(predicts, dtype=np.float64)
    lab = np.asarray(labels).astype(np.int64)
    xl = pred[np.arange(N), lab]
    ce = -xl.sum()
    for c in range(FLIP):
        se = m[c][:, 20:24]  # se[p, cc] = sum_k exp(pred[c*512+cc*128+p, k])
        ce += np.log(se).T.reshape(B).sum()

    val = ALPHA * (-(nce) / 1024.0) + ce / N
    return np.array(val, dtype=np.float32)


def _run_hw(in_maps, trace=False):
    from concourse.bass_utils import run_bass_kernel_spmd

    nc = _get_nc()
    return run_bass_kernel_spmd(nc, in_maps, core_ids=list(range(FLIP)), trace=trace)


def kernel(predicts, labels, features, indexs=None, **_):
    in_maps = _prep_in_maps(predicts, labels, features)
    res = _run_hw(in_maps)
    return _combine(res.results, predicts, labels, features)


def kernel_sim(predicts, labels, features, indexs=None, **_):
    """CoreSim (CPU simulator) path for fast correctness iteration."""
    from concourse.bass_interp import CoreSim

    nc = _get_nc()
    in_maps = _prep_in_maps(predicts, labels, features)
    outs = []
    for a in range(FLIP):
        sim = CoreSim(nc, trace=False)
        for k, v in in_maps[a].items():
            sim.tensor(k)[:] = v
        sim.simulate()
        outs.append({k: np.array(sim.tensor(k)) for k in ("out", "cs1")})
    return _combine(outs, predicts, labels, features)


# revision 23
# speedup vs baseline: 1.0087x; 1.0080x over previous
"""Trainium2 Bass kernel for nn_BatchFlipLoss (NCE batch-flip loss + CE loss).

Math reformulation (validated ~1e-7 vs the jax reference in f64; the
first-order series below adds ~9e-5, vs a 2e-2 gate):

The reference sums BatchCriterion over 36 flip-class pairs (i,j), j>=i.
For pair (i,j) with x = [f_i; f_j] (f_c = features[c::8], L2-normalized,
B=512 rows each), T=0.1, the loss decomposes over ordered halves (a,b).
With E_ab = exp(10*G_ab), G_ab = f_a@f_b.T, S_ab = rowsum(E_ab),
d_ab[p] = f_a[p].f_b[p]:

  D_ab = S0_aa + S_ab      (S0_aa: diag-removed; (a,a): D = 2*S0_aa+e^10)
  half = 10*d - ln(D) - N1/D - ln(1 - exp(10 d)/D),  N1 = S0_aa + S_ab
  (a,a) pair = 2*(10*d - lnD - 2*S0_aa/D)
  ln(1-x) ~ -x only (the x^2/2 tail is ~9e-5 relative after scaling).

Work assignment: 36 unordered blocks over 8 cores = 4.5 each. Core c
computes its diag block (c,c) and blocks (c,c+1), (c,c+2) in full, plus
HALF of its distance-3 and distance-4 blocks: for pair {a, a+k}
(k=3,4), core a takes columns 0:256 of E(f_a rows x f_{a+k} cols) and
core a+k takes rows 256:512 of the mirror block — identical instruction
stream, different host-packed inputs (four [128,256] matmuls per half).
Splitting the late blocks keeps every PSUM colsum chain short so its
staging copy + DMA hides under the final exp groups.

Device pipeline per core (SPMD, inputs rotated so own class is block 0):
  - Gram matmuls write 1-3 tile-slots into multi-bank PSUM tiles; ONE
    ACT exp per group ([128,512..1536]) converts to bf16 E in SBUF.
  - per-slot rowsums: DVE tensor_scalar(+accum_out) on the bf16 E (4x
    DVE mode, accum free) -> out[:, col]; the last group is a single
    diag tile whose rowsum rides the exp's own accumulator, so only ACT
    gates the output DMA.
  - colsums (the partner core's rowsums): PE matmuls with one-hot lhsT
    accumulate j1/j2 chains into a [2,512] PSUM tile (closed mid-kernel)
    and the d3/d4 quarter chains into a [6,256] tile (closed one group
    before the last two cs-free diag groups).
  - CE: one ACT exp over [128,1600] predicts + DVE accum per 400-chunk.
  - diag of block (c,c) is NOT zeroed on device: the host subtracts
    exp(10*||f_p||^2_bf16) from the raw diag rowsums.
The host does only O(N*D)/O(N) work: input layout, d_ab products, the
CE label gather, and the closed-form scalar combine.
"""

from contextlib import ExitStack

import numpy as np

FLIP = 8
B = 512
D = 128
C = 400
N = 4096
ALPHA = 0.03
E10 = float(np.exp(np.float32(10.0)))

# ftp column layout (bf16)
J1 = 512               # 512:1024   distance-1 block
J2 = 1024              # 1024:1536  distance-2 block
R3P0 = 1536            # 1536:2048  d3 P0 rhs (partner[0:256] twice)
R3P1 = 2048            # 2048:2560  d3 P1 rhs (mirror partner, full)
L4 = 2560              # 2560:3072  d4 lhsT chunks (parity-packed)
R4 = 3072              # 3072:3584  d4 rhs halves (parity-packed)
OHJ = 3584             # 3584:3588  one-hots for j1/j2 colsum rows
OH6 = 3588             # 3588:3624  one-hots for the 6 quarter-chain rows
FT_COLS = 3632

# slot ids: s0=(j0,r0) | s1..s4 = j1 r0..r3 | s5..s8 = j2 r0..r3 |
# s9,s10 = d3-P0 halves | s11,s12 = d3-P1 halves | s13,s14 = d4 halves |
# s15..s17 = (j0, r1..r3)
# outt rowsum cols: s0->0, s1..s8 -> 1..8, half-slot quarters -> 9..20
# (two cols per half-slot), j0 r1..r3 -> 21..23, CE -> 24..27.
GROUPS = [[0], [15, 1, 2], [3, 4, 5], [6, 7, 8], [13, 14, 9],
          [10, 11, 12], [16], [17]]
J0_COL = {0: 0, 1: 21, 2: 22, 3: 23}
# half-slot kind -> (first quarter's outt col, cs4 row base, lhs offsets, rhs base)
HALF_KIND = {
    "d3p0": (9, 2, (0, 128, 256, 384), R3P0),
    "d3p1": (13, 4, (256, 384, 256, 384), R3P1),
    "d4": (17, 0, (L4, L4 + 128, L4 + 256, L4 + 384), R4),
}

_CACHE = {}


def _slot_info(s):
    if s == 0:
        return ("full", 0, 0)
    if 1 <= s <= 8:
        return ("full", 1 + (s - 1) // 4, (s - 1) % 4)
    if s in (9, 10):
        return ("d3p0", None, s - 9)
    if s in (11, 12):
        return ("d3p1", None, s - 11)
    if s in (13, 14):
        return ("d4", None, s - 13)
    return ("full", 0, s - 14)


def _build_nc():
    import concourse.tile as tile
    from concourse import bacc, mybir

    f32 = mybir.dt.float32
    bf16 = mybir.dt.bfloat16
    AF = mybir.ActivationFunctionType
    OP = mybir.AluOpType

    nc = bacc.Bacc("TRN2", target_bir_lowering=False, debug=False)

    ft_d = nc.dram_tensor("ft", [D, FT_COLS], bf16, kind="ExternalInput")
    pred_d = nc.dram_tensor("pred", [128, 4 * C], f32, kind="ExternalInput")
    out_d = nc.dram_tensor("out", [128, 28], f32, kind="ExternalOutput")
    csj_d = nc.dram_tensor("csj", [2, B], f32, kind="ExternalOutput")
    cs4_d = nc.dram_tensor("cs4", [6, 256], f32, kind="ExternalOutput")

    with tile.TileContext(nc) as tc, ExitStack() as ctx:
        const = ctx.enter_context(tc.tile_pool(name="const", bufs=1))
        gpool = ctx.enter_context(tc.tile_pool(name="gp", bufs=2, space="PSUM"))
        cjpool = ctx.enter_context(tc.tile_pool(name="cj", bufs=1, space="PSUM"))
        c4pool = ctx.enter_context(tc.tile_pool(name="c4", bufs=1, space="PSUM"))
        epool = ctx.enter_context(tc.tile_pool(name="ep", bufs=4))
        small = ctx.enter_context(tc.tile_pool(name="sm", bufs=1))

        ftt = const.tile([D, FT_COLS], bf16)
        predt = const.tile([128, 4 * C], f32)
        outt = small.tile([128, 28], f32)
        scr = small.tile([128, 3 * B], bf16)
        scrp = small.tile([128, C], bf16)
        csjs = small.tile([2, B], f32)
        cs4s = small.tile([6, 256], f32)

        nc.sync.dma_start(ftt[:, 0:B], ft_d[:, 0:B])
        nc.sync.dma_start(ftt[:, B : 2 * B], ft_d[:, B : 2 * B])
        nc.sync.dma_start(ftt[:, 2 * B : FT_COLS], ft_d[:, 2 * B : FT_COLS])
        nc.sync.dma_start(predt[:], pred_d[:, :])

        csjp = cjpool.tile([2, B], f32)
        cs4p = c4pool.tile([6, 256], f32)

        pg = {}
        eg = {}

        def emit_mms(g):
            slots = GROUPS[g]
            pgt = gpool.tile([128, len(slots) * B], f32, tag="pg")
            pg[g] = pgt
            for i, s in enumerate(slots):
                kind, j, r = _slot_info(s)
                if kind == "full":
                    nc.tensor.matmul(
                        pgt[:, i * B : (i + 1) * B],
                        ftt[:, r * 128 : (r + 1) * 128],
                        ftt[:, j * B : (j + 1) * B],
                        start=True,
                        stop=True,
                    )
                else:
                    _, _, lhs_off, rhs_base = HALF_KIND[kind]
                    for h in range(2):
                        q = 2 * r + h
                        lo = lhs_off[q] if kind != "d3p0" else 128 * q
                        nc.tensor.matmul(
                            pgt[:, i * B + h * 256 : i * B + (h + 1) * 256],
                            ftt[:, lo : lo + 128],
                            ftt[:, rhs_base + (q // 2) * 256 : rhs_base + (q // 2 + 1) * 256],
                            start=True,
                            stop=True,
                        )

        def emit_exp(g, accum=None):
            egt = epool.tile([128, len(GROUPS[g]) * B], bf16, tag="eg")
            eg[g] = egt
            nc.scalar.activation(
                egt[:], pg[g][:], AF.Exp, bias=0.0, scale=10.0, accum_out=accum
            )

        def emit_rs(g):
            for i, s in enumerate(GROUPS[g]):
                kind, j, r = _slot_info(s)
                if kind == "full":
                    col = J0_COL[r] if j == 0 else s
                    nc.vector.tensor_scalar(
                        scr[:, i * B : (i + 1) * B],
                        eg[g][:, i * B : (i + 1) * B],
                        1.0, None, OP.mult, OP.add,
                        accum_out=outt[:, col : col + 1],
                    )
                else:
                    col0 = HALF_KIND[kind][0]
                    for h in range(2):
                        col = col0 + 2 * r + h
                        nc.vector.tensor_scalar(
                            scr[:, i * B + h * 256 : i * B + (h + 1) * 256],
                            eg[g][:, i * B + h * 256 : i * B + (h + 1) * 256],
                            1.0, None, OP.mult, OP.add,
                            accum_out=outt[:, col : col + 1],
                        )

        def emit_cs(g):
            for i, s in enumerate(GROUPS[g]):
                kind, j, r = _slot_info(s)
                if kind == "full":
                    if j not in (1, 2):
                        continue
                    oh = OHJ + 2 * (j - 1)
                    nc.tensor.matmul(
                        csjp[:],
                        ftt[:, oh : oh + 2],
                        eg[g][:, i * B : (i + 1) * B],
                        start=(s == 1),
                        stop=(s == 8),
                    )
                else:
                    row_base = HALF_KIND[kind][1]
                    for h in range(2):
                        q = 2 * r + h
                        row = row_base + q // 2
                        oh = OH6 + 6 * row
                        nc.tensor.matmul(
                            cs4p[:],
                            ftt[:, oh : oh + 6],
                            eg[g][:, i * B + h * 256 : i * B + (h + 1) * 256],
                            start=(s == 13 and q == 0),
                            stop=(s == 12 and q == 3),
                        )

        emit_mms(0)
        emit_mms(1)
        emit_exp(0)
        emit_rs(0)
        for g in range(2, len(GROUPS)):
            emit_mms(g)
            emit_exp(g - 1)
            if g == 5:
                # j1/j2 colsum chain closed by cs(3); stage + DMA while DVE
                # is otherwise idle, well before the pipeline tails
                emit_cs(g - 2)
                nc.vector.tensor_copy(csjs[:], csjp[:])
                nc.gpsimd.dma_start(csj_d[:, :], csjs[:])
                emit_rs(g - 1)
            elif g == 7:
                # quarter-chain (d3/d4) closed by cs(5): small [6,256] copy
                # slots into DVE slack ahead of rs(6); its Pool-queue DMA
                # overlaps the last two exps
                emit_cs(g - 2)
                nc.vector.tensor_copy(cs4s[:], cs4p[:])
                nc.gpsimd.dma_start(cs4_d[:, :], cs4s[:])
                emit_rs(g - 1)
            else:
                emit_rs(g - 1)
                if g >= 3:
                    emit_cs(g - 2)
            if g == 4:
                # CE: predicts arrive behind ft; slot the exp mid-pipeline.
                ept = epool.tile([128, 4 * C], bf16, tag="ept")
                nc.scalar.activation(ept[:], predt[:], AF.Exp, bias=0.0, scale=1.0)
                for cchunk in range(4):
                    nc.vector.tensor_scalar(
                        scrp[:],
                        ept[:, cchunk * C : (cchunk + 1) * C],
                        1.0, None, OP.mult, OP.add,
                        accum_out=outt[:, 24 + cchunk : 25 + cchunk],
                    )
        # last group: single diag tile, rowsum via the exp's accumulator
        gl = len(GROUPS) - 1
        emit_exp(gl, accum=outt[:, 23:24])

        nc.sync.dma_start(out_d[:, :], outt[:])

    nc.compile()
    return nc


def _get_nc():
    if "nc" not in _CACHE:
        _CACHE["nc"] = _build_nc()
    return _CACHE["nc"]


def _prep_in_maps(predicts, labels, features):
    import ml_dtypes

    feats = np.ascontiguousarray(features, dtype=np.float32)
    pred = np.ascontiguousarray(predicts, dtype=np.float32)
    f8 = feats.reshape(B, FLIP, D).transpose(1, 0, 2)  # [8,512,128], f8[c]=feats[c::8]

    ohj = np.zeros((D, 4), dtype=np.float32)
    ohj[:, 0] = 1.0   # j1 -> csj row 0
    ohj[:, 3] = 1.0   # j2 -> csj row 1
    oh6 = np.zeros((D, 36), dtype=np.float32)
    for r in range(6):
        oh6[:, 6 * r + r] = 1.0

    in_maps = []
    for a in range(FLIP):
        ft = np.zeros((D, FT_COLS), dtype=np.float32)
        ft[:, 0:B] = f8[a].T
        ft[:, J1 : J1 + B] = f8[(a + 1) % FLIP].T
        ft[:, J2 : J2 + B] = f8[(a + 2) % FLIP].T
        p3 = f8[(a + 3) % FLIP]
        ft[:, R3P0 : R3P0 + 256] = p3[0:256].T
        ft[:, R3P0 + 256 : R3P1] = p3[0:256].T
        ft[:, R3P1 : R3P1 + B] = f8[(a - 3) % FLIP].T
        p4 = f8[(a + 4) % FLIP]
        own = f8[a]
        if a < 4:
            ft[:, L4 : L4 + B] = own.T
            ft[:, R4 : R4 + 256] = p4[0:256].T
            ft[:, R4 + 256 : R4 + 512] = p4[0:256].T
        else:
            ft[:, L4 : L4 + 128] = own[256:384].T
            ft[:, L4 + 128 : L4 + 256] = own[384:512].T
            ft[:, L4 + 256 : L4 + 384] = own[256:384].T
            ft[:, L4 + 384 : R4] = own[384:512].T
            ft[:, R4 : R4 + B] = p4.T
        ft[:, OHJ:OH6] = ohj
        ft[:, OH6 : OH6 + 36] = oh6
        pb = pred[a * B : (a + 1) * B].reshape(4, 128, C).transpose(1, 0, 2)
        in_maps.append(
            {
                "ft": np.ascontiguousarray(ft).astype(ml_dtypes.bfloat16),
                "pred": np.ascontiguousarray(pb.reshape(128, 4 * C)),
            }
        )
    return in_maps


def _stitch_pair(mP0, mP1, csP0, csP1, colP0, colP1, rowP0, rowP1):
    """Assemble both rowsum directions of a split block M (P0 core holds
    cols 0:256 over all rows; P1 core holds rows 256:512 over all cols)."""
    partial = mP0[:, colP0 : colP0 + 4].T.reshape(B)      # cols 0:256, by chunk
    compl_ = np.concatenate([csP1[rowP1], csP1[rowP1 + 1]])  # cols 256:512
    s_fwd = partial + compl_
    lo = csP0[rowP0] + csP0[rowP0 + 1]                    # mirror rows 0:256
    hi = np.empty(256)
    hi[0:128] = mP1[:, colP1] + mP1[:, colP1 + 2]         # rows 256:384
    hi[128:256] = mP1[:, colP1 + 1] + mP1[:, colP1 + 3]   # rows 384:512
    s_rev = np.concatenate([lo, hi])
    return s_fwd, s_rev


def _combine(outs, predicts, labels, features):
    """Host-side O(N*D) combine: reroute per-block sums between the
    ordered halves and apply the closed-form first-order series."""
    import ml_dtypes

    feats = np.asarray(features, dtype=np.float32)
    f8 = feats.reshape(B, FLIP, D).transpose(1, 0, 2).astype(np.float64)
    fb8 = f8.astype(ml_dtypes.bfloat16).astype(np.float64)  # device-side values

    dv = np.einsum("apd,bpd->abp", f8, f8)

    m = {}
    csj = {}
    cs4 = {}
    for c in range(FLIP):
        m[c] = np.asarray(outs[c]["out"], np.float64)
        csj[c] = np.asarray(outs[c]["csj"], np.float64)
        cs4[c] = np.asarray(outs[c]["cs4"], np.float64)

    S1 = {}
    for c in range(FLIP):
        for j in range(3):
            b = (c + j) % FLIP
            cols = [J0_COL[r] for r in range(4)] if j == 0 else [1 + 4 * (j - 1) + r for r in range(4)]
            S1[(c, b)] = m[c][:, cols].T.reshape(B)
        for j in (1, 2):
            S1[((c + j) % FLIP, c)] = csj[c][j - 1]

    for b in range(FLIP):  # distance-3 pairs, P0 = core b, P1 = core b+3
        bp = (b + 3) % FLIP
        s_fwd, s_rev = _stitch_pair(m[b], m[bp], cs4[b], cs4[bp], 9, 13, 2, 4)
        S1[(b, bp)] = s_fwd
        S1[(bp, b)] = s_rev
    for b in range(4):     # distance-4 pairs, P0 = core b, P1 = core b+4
        bp = b + 4
        s_fwd, s_rev = _stitch_pair(m[b], m[bp], cs4[b], cs4[bp], 17, 17, 0, 0)
        S1[(b, bp)] = s_fwd
        S1[(bp, b)] = s_rev

    # remove the raw diagonal exp from the own-block rowsums.  chunks r0-r2
    # were summed from bf16 E by DVE; chunk r3 rides the ACT accumulator
    # (f32 activation results), so skip the bf16 rounding there.
    S10 = {}
    for c in range(FLIP):
        gpp = np.einsum("pd,pd->p", fb8[c], fb8[c])
        dg = np.exp(10.0 * gpp).astype(np.float32)
        dgb = dg.astype(ml_dtypes.bfloat16).astype(np.float64)
        dgb[384:512] = dg[384:512]
        S10[c] = S1[(c, c)] - dgb

    nce = 0.0
    for a in range(FLIP):
        for b in range(FLIP):
            d = dv[a, b]
            if a == b:
                N1 = 2.0 * S10[a]
                Dv = N1 + E10
                half = 10.0 * d - np.log(Dv) - N1 / Dv
                nce += 2.0 * half.sum()
            else:
                N1 = S10[a] + S1[(a, b)]
                half = (
                    10.0 * d
                    - np.log(N1)
                    - 1.0
                    - np.log1p(-np.exp(10.0 * d) / N1)
                )
                nce += half.sum()

    # CE: device exp-sums + host label gather
    pred = np.asarray(predicts, dtype=np.float64)
    lab = np.asarray(labels).astype(np.int64)
    xl = pred[np.arange(N), lab]
    ce = -xl.sum()
    for c in range(FLIP):
        se = m[c][:, 24:28]  # se[p, cc] = sum_k exp(pred[c*512+cc*128+p, k])
        ce += np.log(se).T.reshape(B).sum()

    val = ALPHA * (-(nce) / 1024.0) + ce / N
    return np.array(val, dtype=np.float32)


def _run_hw(in_maps, trace=False):
    from concourse.bass_utils import run_bass_kernel_spmd

    nc = _get_nc()
    return run_bass_kernel_spmd(nc, in_maps, core_ids=list(range(FLIP)), trace=trace)


def kernel(predicts, labels, features, indexs=None, **_):
    in_maps = _prep_in_maps(predicts, labels, features)
    res = _run_hw(in_maps)
    return _combine(res.results, predicts, labels, features)


def kernel_sim(predicts, labels, features, indexs=None, **_):
    """CoreSim (CPU simulator) path for fast correctness iteration."""
    from concourse.bass_interp import CoreSim

    nc = _get_nc()
    in_maps = _prep_in_maps(predicts, labels, features)
    outs = []
    for a in range(FLIP):
        sim = CoreSim(nc, trace=False)
        for k, v in in_maps[a].items():
            sim.tensor(k)[:] = v
        sim.simulate()
        outs.append({k: np.array(sim.tensor(k)) for k in ("out", "csj", "cs4")})
    return _combine(outs, predicts, labels, features)


# revision 32
# speedup vs baseline: 1.0348x; 1.0259x over previous
"""Trainium2 Bass kernel for nn_BatchFlipLoss (NCE batch-flip loss + CE loss).

Math reformulation (validated ~1e-7 vs the jax reference in f64; the
first-order series below adds ~9e-5, vs a 2e-2 gate):

The reference sums BatchCriterion over 36 flip-class pairs (i,j), j>=i.
For pair (i,j) with x = [f_i; f_j] (f_c = features[c::8], L2-normalized,
B=512 rows each), T=0.1, the loss decomposes over ordered halves (a,b).
With E_ab = exp(10*G_ab), G_ab = f_a@f_b.T, S_ab = rowsum(E_ab),
d_ab[p] = f_a[p].f_b[p]:

  D_ab = S0_aa + S_ab      (S0_aa: diag-removed; (a,a): D = 2*S0_aa+e^10)
  half = 10*d - ln(D) - N1/D - ln(1 - exp(10 d)/D),  N1 = S0_aa + S_ab
  (a,a) pair = 2*(10*d - lnD - 2*S0_aa/D)
  ln(1-x) ~ -x only (the x^2/2 tail is ~9e-5 relative after scaling).

Work assignment: 36 unordered blocks over 8 cores = 4.5 each. Core c
computes its diag block (c,c) and blocks (c,c+1), (c,c+2) in full, plus
HALF of its distance-3 and distance-4 blocks: for pair {a, a+k}
(k=3,4), core a takes columns 0:256 of E(f_a rows x f_{a+k} cols) and
core a+k takes rows 256:512 of the mirror block — identical instruction
stream, different host-packed inputs (four [128,256] matmuls per half).
Splitting the late blocks keeps every PSUM colsum chain short so its
staging copy + DMA hides under the final exp groups.

Device pipeline per core (SPMD, inputs rotated so own class is block 0):
  - Gram matmuls write 1-3 tile-slots into multi-bank PSUM tiles; ONE
    ACT exp per group ([128,512..1536]) converts to bf16 E in SBUF.
  - per-slot rowsums: DVE tensor_scalar(+accum_out) on the bf16 E (4x
    DVE mode, accum free) -> out[:, col]; the last group is a single
    diag tile whose rowsum rides the exp's own accumulator, so only ACT
    gates the output DMA.
  - colsums (the partner core's rowsums): PE matmuls with one-hot lhsT
    accumulate j1/j2 chains into a [2,512] PSUM tile (closed mid-kernel)
    and the d3/d4 quarter chains into a [6,256] tile (closed one group
    before the last two cs-free diag groups).
  - CE: one ACT exp over [128,1600] predicts + DVE accum per 400-chunk.
  - diag of block (c,c) is NOT zeroed on device: the host subtracts
    exp(10*||f_p||^2_bf16) from the raw diag rowsums.
The host does only O(N*D)/O(N) work: input layout, d_ab products, the
CE label gather, and the closed-form scalar combine.
"""

from contextlib import ExitStack

import numpy as np

FLIP = 8
B = 512
D = 128
C = 400
N = 4096
ALPHA = 0.03
E10 = float(np.exp(np.float32(10.0)))

# ftp column layout (bf16)
J1 = 512               # 512:1024   distance-1 block
J2 = 1024              # 1024:1536  distance-2 block
R3P0 = 1536            # 1536:2048  d3 P0 rhs (partner[0:256] twice)
R3P1 = 2048            # 2048:2560  d3 P1 rhs (mirror partner, full)
L4 = 2560              # 2560:3072  d4 lhsT chunks (parity-packed)
R4 = 3072              # 3072:3584  d4 rhs halves (parity-packed)
OHJ = 3584             # 3584:3588  one-hots for j1/j2 colsum rows
OH6 = 3588             # 3588:3624  one-hots for the 6 quarter-chain rows
FT_COLS = 3632

# slot ids: s0=(j0,r0) | s1..s4 = j1 r0..r3 | s5..s8 = j2 r0..r3 |
# s9,s10 = d3-P0 halves | s11,s12 = d3-P1 halves | s13,s14 = d4 halves |
# s15..s17 = (j0, r1..r3)
# outt rowsum cols: s0->0, s1..s8 -> 1..8, half-slot quarters -> 9..20
# (two cols per half-slot), j0 r1..r3 -> 21..23, CE -> 24..27.
GROUPS = [[0], [15, 1, 2], [3, 4, 5], [6, 7, 8], [13, 14, 9],
          [10, 11, 12], [16], [17]]
J0_COL = {0: 0, 1: 21, 2: 22, 3: 23}
# half-slot kind -> (first quarter's outt col, cs4 row base, lhs offsets, rhs base)
HALF_KIND = {
    "d3p0": (9, 2, (0, 128, 256, 384), R3P0),
    "d3p1": (13, 4, (256, 384, 256, 384), R3P1),
    "d4": (17, 0, (L4, L4 + 128, L4 + 256, L4 + 384), R4),
}

_CACHE = {}


def _slot_info(s):
    if s == 0:
        return ("full", 0, 0)
    if 1 <= s <= 8:
        return ("full", 1 + (s - 1) // 4, (s - 1) % 4)
    if s in (9, 10):
        return ("d3p0", None, s - 9)
    if s in (11, 12):
        return ("d3p1", None, s - 11)
    if s in (13, 14):
        return ("d4", None, s - 13)
    return ("full", 0, s - 14)


def _build_nc():
    import concourse.tile as tile
    from concourse import bacc, mybir

    f32 = mybir.dt.float32
    bf16 = mybir.dt.bfloat16
    AF = mybir.ActivationFunctionType
    OP = mybir.AluOpType

    nc = bacc.Bacc("TRN2", target_bir_lowering=False, debug=False)

    ft_d = nc.dram_tensor("ft", [D, FT_COLS], bf16, kind="ExternalInput")
    pred_d = nc.dram_tensor("pred", [128, 4 * C], f32, kind="ExternalInput")
    out_d = nc.dram_tensor("out", [128, 28], f32, kind="ExternalOutput")
    csj_d = nc.dram_tensor("csj", [2, B], f32, kind="ExternalOutput")
    cs4_d = nc.dram_tensor("cs4", [6, 256], f32, kind="ExternalOutput")

    with tile.TileContext(nc) as tc, ExitStack() as ctx:
        const = ctx.enter_context(tc.tile_pool(name="const", bufs=1))
        gpool = ctx.enter_context(tc.tile_pool(name="gp", bufs=2, space="PSUM"))
        cjpool = ctx.enter_context(tc.tile_pool(name="cj", bufs=1, space="PSUM"))
        c4pool = ctx.enter_context(tc.tile_pool(name="c4", bufs=1, space="PSUM"))
        epool = ctx.enter_context(tc.tile_pool(name="ep", bufs=4))
        small = ctx.enter_context(tc.tile_pool(name="sm", bufs=1))

        ftt = const.tile([D, FT_COLS], bf16)
        predt = const.tile([128, 4 * C], f32)
        outt = small.tile([128, 28], f32)
        scr = small.tile([128, 3 * B], bf16)
        scrp = small.tile([128, C], bf16)
        csjs = small.tile([2, B], f32)
        cs4s = small.tile([6, 256], f32)

        nc.sync.dma_start(ftt[:, 0:B], ft_d[:, 0:B])
        nc.sync.dma_start(ftt[:, B : 2 * B], ft_d[:, B : 2 * B])
        nc.sync.dma_start(ftt[:, 2 * B : 3 * B], ft_d[:, 2 * B : 3 * B])
        nc.sync.dma_start(ftt[:, 3 * B : FT_COLS], ft_d[:, 3 * B : FT_COLS])
        nc.sync.dma_start(predt[:], pred_d[:, :])

        # one shared bank: j1/j2 colsums live in partitions 0:2 until their
        # staging copy; the final group's Gram tile then overwrites it
        cshp = cjpool.tile([128, B], f32)
        cs4p = c4pool.tile([6, 256], f32)

        pg = {}
        eg = {}

        def emit_mms(g, tile_=None):
            slots = GROUPS[g]
            if tile_ is None:
                pgt = gpool.tile([128, len(slots) * B], f32, tag="pg")
            else:
                pgt = tile_
            pg[g] = pgt
            for i, s in enumerate(slots):
                kind, j, r = _slot_info(s)
                if kind == "full":
                    nc.tensor.matmul(
                        pgt[:, i * B : (i + 1) * B],
                        ftt[:, r * 128 : (r + 1) * 128],
                        ftt[:, j * B : (j + 1) * B],
                        start=True,
                        stop=True,
                    )
                else:
                    _, _, lhs_off, rhs_base = HALF_KIND[kind]
                    for h in range(2):
                        q = 2 * r + h
                        lo = lhs_off[q] if kind != "d3p0" else 128 * q
                        nc.tensor.matmul(
                            pgt[:, i * B + h * 256 : i * B + (h + 1) * 256],
                            ftt[:, lo : lo + 128],
                            ftt[:, rhs_base + (q // 2) * 256 : rhs_base + (q // 2 + 1) * 256],
                            start=True,
                            stop=True,
                        )

        def emit_exp(g, accum=None):
            egt = epool.tile([128, len(GROUPS[g]) * B], bf16, tag="eg")
            eg[g] = egt
            nc.scalar.activation(
                egt[:], pg[g][:], AF.Exp, bias=0.0, scale=10.0, accum_out=accum
            )

        def emit_rs(g):
            for i, s in enumerate(GROUPS[g]):
                kind, j, r = _slot_info(s)
                if kind == "full":
                    col = J0_COL[r] if j == 0 else s
                    nc.vector.tensor_scalar(
                        scr[:, i * B : (i + 1) * B],
                        eg[g][:, i * B : (i + 1) * B],
                        1.0, None, OP.mult, OP.add,
                        accum_out=outt[:, col : col + 1],
                    )
                else:
                    col0 = HALF_KIND[kind][0]
                    for h in range(2):
                        col = col0 + 2 * r + h
                        nc.vector.tensor_scalar(
                            scr[:, i * B + h * 256 : i * B + (h + 1) * 256],
                            eg[g][:, i * B + h * 256 : i * B + (h + 1) * 256],
                            1.0, None, OP.mult, OP.add,
                            accum_out=outt[:, col : col + 1],
                        )

        def emit_cs(g):
            for i, s in enumerate(GROUPS[g]):
                kind, j, r = _slot_info(s)
                if kind == "full":
                    if j not in (1, 2):
                        continue
                    oh = OHJ + 2 * (j - 1)
                    nc.tensor.matmul(
                        cshp[0:2, :],
                        ftt[:, oh : oh + 2],
                        eg[g][:, i * B : (i + 1) * B],
                        start=(s == 1),
                        stop=(s == 8),
                    )
                else:
                    row_base = HALF_KIND[kind][1]
                    for h in range(2):
                        q = 2 * r + h
                        row = row_base + q // 2
                        oh = OH6 + 6 * row
                        nc.tensor.matmul(
                            cs4p[:],
                            ftt[:, oh : oh + 6],
                            eg[g][:, i * B + h * 256 : i * B + (h + 1) * 256],
                            start=(s == 13 and q == 0),
                            stop=(s == 12 and q == 3),
                        )

        emit_mms(0)
        emit_mms(1)
        emit_exp(0)
        emit_rs(0)
        for g in range(2, len(GROUPS)):
            # the final group reuses the retired j1/j2-colsum bank so its
            # matmul isn't stuck waiting for the rotating pipeline banks
            emit_mms(g, tile_=cshp if g == len(GROUPS) - 1 else None)
            emit_exp(g - 1)
            if g == 5:
                # j1/j2 colsum chain closed by cs(3); stage + DMA while DVE
                # is otherwise idle, well before the pipeline tails
                emit_cs(g - 2)
                nc.vector.tensor_copy(csjs[:], cshp[0:2, :])
                nc.gpsimd.dma_start(csj_d[:, :], csjs[:])
                emit_rs(g - 1)
            elif g == 7:
                # quarter-chain (d3/d4) closed by cs(5): small [6,256] copy
                # slots into DVE slack ahead of rs(6); its Pool-queue DMA
                # overlaps the last two exps
                emit_cs(g - 2)
                nc.vector.tensor_copy(cs4s[:], cs4p[:])
                nc.gpsimd.dma_start(cs4_d[:, :], cs4s[:])
                emit_rs(g - 1)
            else:
                emit_rs(g - 1)
                if g >= 3:
                    emit_cs(g - 2)
            if g == 4:
                # CE: predicts arrive behind ft; slot the exp mid-pipeline.
                ept = epool.tile([128, 4 * C], bf16, tag="ept")
                nc.scalar.activation(ept[:], predt[:], AF.Exp, bias=0.0, scale=1.0)
                for cchunk in range(4):
                    nc.vector.tensor_scalar(
                        scrp[:],
                        ept[:, cchunk * C : (cchunk + 1) * C],
                        1.0, None, OP.mult, OP.add,
                        accum_out=outt[:, 24 + cchunk : 25 + cchunk],
                    )
        # last group: single diag tile, rowsum via the exp's accumulator
        gl = len(GROUPS) - 1
        emit_exp(gl, accum=outt[:, 23:24])

        nc.sync.dma_start(out_d[:, :], outt[:])

    nc.compile()
    return nc


def _get_nc():
    if "nc" not in _CACHE:
        _CACHE["nc"] = _build_nc()
    return _CACHE["nc"]


def _prep_in_maps(predicts, labels, features):
    import ml_dtypes

    feats = np.ascontiguousarray(features, dtype=np.float32)
    pred = np.ascontiguousarray(predicts, dtype=np.float32)
    f8 = feats.reshape(B, FLIP, D).transpose(1, 0, 2)  # [8,512,128], f8[c]=feats[c::8]

    ohj = np.zeros((D, 4), dtype=np.float32)
    ohj[:, 0] = 1.0   # j1 -> csj row 0
    ohj[:, 3] = 1.0   # j2 -> csj row 1
    oh6 = np.zeros((D, 36), dtype=np.float32)
    for r in range(6):
        oh6[:, 6 * r + r] = 1.0

    in_maps = []
    for a in range(FLIP):
        ft = np.zeros((D, FT_COLS), dtype=np.float32)
        ft[:, 0:B] = f8[a].T
        ft[:, J1 : J1 + B] = f8[(a + 1) % FLIP].T
        ft[:, J2 : J2 + B] = f8[(a + 2) % FLIP].T
        p3 = f8[(a + 3) % FLIP]
        ft[:, R3P0 : R3P0 + 256] = p3[0:256].T
        ft[:, R3P0 + 256 : R3P1] = p3[0:256].T
        ft[:, R3P1 : R3P1 + B] = f8[(a - 3) % FLIP].T
        p4 = f8[(a + 4) % FLIP]
        own = f8[a]
        if a < 4:
            ft[:, L4 : L4 + B] = own.T
            ft[:, R4 : R4 + 256] = p4[0:256].T
            ft[:, R4 + 256 : R4 + 512] = p4[0:256].T
        else:
            ft[:, L4 : L4 + 128] = own[256:384].T
            ft[:, L4 + 128 : L4 + 256] = own[384:512].T
            ft[:, L4 + 256 : L4 + 384] = own[256:384].T
            ft[:, L4 + 384 : R4] = own[384:512].T
            ft[:, R4 : R4 + B] = p4.T
        ft[:, OHJ:OH6] = ohj
        ft[:, OH6 : OH6 + 36] = oh6
        pb = pred[a * B : (a + 1) * B].reshape(4, 128, C).transpose(1, 0, 2)
        in_maps.append(
            {
                "ft": np.ascontiguousarray(ft).astype(ml_dtypes.bfloat16),
                "pred": np.ascontiguousarray(pb.reshape(128, 4 * C)),
            }
        )
    return in_maps


def _stitch_pair(mP0, mP1, csP0, csP1, colP0, colP1, rowP0, rowP1):
    """Assemble both rowsum directions of a split block M (P0 core holds
    cols 0:256 over all rows; P1 core holds rows 256:512 over all cols)."""
    partial = mP0[:, colP0 : colP0 + 4].T.reshape(B)      # cols 0:256, by chunk
    compl_ = np.concatenate([csP1[rowP1], csP1[rowP1 + 1]])  # cols 256:512
    s_fwd = partial + compl_
    lo = csP0[rowP0] + csP0[rowP0 + 1]                    # mirror rows 0:256
    hi = np.empty(256)
    hi[0:128] = mP1[:, colP1] + mP1[:, colP1 + 2]         # rows 256:384
    hi[128:256] = mP1[:, colP1 + 1] + mP1[:, colP1 + 3]   # rows 384:512
    s_rev = np.concatenate([lo, hi])
    return s_fwd, s_rev


def _combine(outs, predicts, labels, features):
    """Host-side O(N*D) combine: reroute per-block sums between the
    ordered halves and apply the closed-form first-order series."""
    import ml_dtypes

    feats = np.asarray(features, dtype=np.float32)
    f8 = feats.reshape(B, FLIP, D).transpose(1, 0, 2).astype(np.float64)
    fb8 = f8.astype(ml_dtypes.bfloat16).astype(np.float64)  # device-side values

    dv = np.einsum("apd,bpd->abp", f8, f8)

    m = {}
    csj = {}
    cs4 = {}
    for c in range(FLIP):
        m[c] = np.asarray(outs[c]["out"], np.float64)
        csj[c] = np.asarray(outs[c]["csj"], np.float64)
        cs4[c] = np.asarray(outs[c]["cs4"], np.float64)

    S1 = {}
    for c in range(FLIP):
        for j in range(3):
            b = (c + j) % FLIP
            cols = [J0_COL[r] for r in range(4)] if j == 0 else [1 + 4 * (j - 1) + r for r in range(4)]
            S1[(c, b)] = m[c][:, cols].T.reshape(B)
        for j in (1, 2):
            S1[((c + j) % FLIP, c)] = csj[c][j - 1]

    for b in range(FLIP):  # distance-3 pairs, P0 = core b, P1 = core b+3
        bp = (b + 3) % FLIP
        s_fwd, s_rev = _stitch_pair(m[b], m[bp], cs4[b], cs4[bp], 9, 13, 2, 4)
        S1[(b, bp)] = s_fwd
        S1[(bp, b)] = s_rev
    for b in range(4):     # distance-4 pairs, P0 = core b, P1 = core b+4
        bp = b + 4
        s_fwd, s_rev = _stitch_pair(m[b], m[bp], cs4[b], cs4[bp], 17, 17, 0, 0)
        S1[(b, bp)] = s_fwd
        S1[(bp, b)] = s_rev

    # remove the raw diagonal exp from the own-block rowsums.  chunks r0-r2
    # were summed from bf16 E by DVE; chunk r3 rides the ACT accumulator
    # (f32 activation results), so skip the bf16 rounding there.
    S10 = {}
    for c in range(FLIP):
        gpp = np.einsum("pd,pd->p", fb8[c], fb8[c])
        dg = np.exp(10.0 * gpp).astype(np.float32)
        dgb = dg.astype(ml_dtypes.bfloat16).astype(np.float64)
        dgb[384:512] = dg[384:512]
        S10[c] = S1[(c, c)] - dgb

    nce = 0.0
    for a in range(FLIP):
        for b in range(FLIP):
            d = dv[a, b]
            if a == b:
                N1 = 2.0 * S10[a]
                Dv = N1 + E10
                half = 10.0 * d - np.log(Dv) - N1 / Dv
                nce += 2.0 * half.sum()
            else:
                N1 = S10[a] + S1[(a, b)]
                half = (
                    10.0 * d
                    - np.log(N1)
                    - 1.0
                    - np.log1p(-np.exp(10.0 * d) / N1)
                )
                nce += half.sum()

    # CE: device exp-sums + host label gather
    pred = np.asarray(predicts, dtype=np.float64)
    lab = np.asarray(labels).astype(np.int64)
    xl = pred[np.arange(N), lab]
    ce = -xl.sum()
    for c in range(FLIP):
        se = m[c][:, 24:28]  # se[p, cc] = sum_k exp(pred[c*512+cc*128+p, k])
        ce += np.log(se).T.reshape(B).sum()

    val = ALPHA * (-(nce) / 1024.0) + ce / N
    return np.array(val, dtype=np.float32)


def _run_hw(in_maps, trace=False):
    from concourse.bass_utils import run_bass_kernel_spmd

    nc = _get_nc()
    return run_bass_kernel_spmd(nc, in_maps, core_ids=list(range(FLIP)), trace=trace)


def kernel(predicts, labels, features, indexs=None, **_):
    in_maps = _prep_in_maps(predicts, labels, features)
    res = _run_hw(in_maps)
    return _combine(res.results, predicts, labels, features)


def kernel_sim(predicts, labels, features, indexs=None, **_):
    """CoreSim (CPU simulator) path for fast correctness iteration."""
    from concourse.bass_interp import CoreSim

    nc = _get_nc()
    in_maps = _prep_in_maps(predicts, labels, features)
    outs = []
    for a in range(FLIP):
        sim = CoreSim(nc, trace=False)
        for k, v in in_maps[a].items():
            sim.tensor(k)[:] = v
        sim.simulate()
        outs.append({k: np.array(sim.tensor(k)) for k in ("out", "csj", "cs4")})
    return _combine(outs, predicts, labels, features)


# revision 33
# speedup vs baseline: 1.0691x; 1.0331x over previous
"""Trainium2 Bass kernel for nn_BatchFlipLoss (NCE batch-flip loss + CE loss).

Math reformulation (validated ~1e-7 vs the jax reference in f64; the
first-order series below adds ~9e-5, vs a 2e-2 gate):

The reference sums BatchCriterion over 36 flip-class pairs (i,j), j>=i.
For pair (i,j) with x = [f_i; f_j] (f_c = features[c::8], L2-normalized,
B=512 rows each), T=0.1, the loss decomposes over ordered halves (a,b).
With E_ab = exp(10*G_ab), G_ab = f_a@f_b.T, S_ab = rowsum(E_ab),
d_ab[p] = f_a[p].f_b[p]:

  D_ab = S0_aa + S_ab      (S0_aa: diag-removed; (a,a): D = 2*S0_aa+e^10)
  half = 10*d - ln(D) - N1/D - ln(1 - exp(10 d)/D),  N1 = S0_aa + S_ab
  (a,a) pair = 2*(10*d - lnD - 2*S0_aa/D)
  ln(1-x) ~ -x only (the x^2/2 tail is ~9e-5 relative after scaling).

Work assignment: 36 unordered blocks over 8 cores = 4.5 each. Core c
computes its diag block (c,c) and blocks (c,c+1), (c,c+2) in full, plus
HALF of its distance-3 and distance-4 blocks: for pair {a, a+k}
(k=3,4), core a takes columns 0:256 of E(f_a rows x f_{a+k} cols) and
core a+k takes rows 256:512 of the mirror block — identical instruction
stream, different host-packed inputs (four [128,256] matmuls per half).
Splitting the late blocks keeps every PSUM colsum chain short so its
staging copy + DMA hides under the final exp groups.

Device pipeline per core (SPMD, inputs rotated so own class is block 0):
  - Gram matmuls write 1-3 tile-slots into multi-bank PSUM tiles; ONE
    ACT exp per group ([128,512..1536]) converts to bf16 E in SBUF.
  - per-slot rowsums: DVE tensor_scalar(+accum_out) on the bf16 E (4x
    DVE mode, accum free) -> out[:, col]; the last group is a single
    diag tile whose rowsum rides the exp's own accumulator, so only ACT
    gates the output DMA.
  - colsums (the partner core's rowsums): PE matmuls with one-hot lhsT
    accumulate j1/j2 chains into a [2,512] PSUM tile (closed mid-kernel)
    and the d3/d4 quarter chains into a [6,256] tile (closed one group
    before the last two cs-free diag groups).
  - CE: one ACT exp over [128,1600] predicts + DVE accum per 400-chunk.
  - diag of block (c,c) is NOT zeroed on device: the host subtracts
    exp(10*||f_p||^2_bf16) from the raw diag rowsums.
The host does only O(N*D)/O(N) work: input layout, d_ab products, the
CE label gather, and the closed-form scalar combine.
"""

from contextlib import ExitStack

import numpy as np

FLIP = 8
B = 512
D = 128
C = 400
N = 4096
ALPHA = 0.03
E10 = float(np.exp(np.float32(10.0)))

# ftp column layout (bf16)
J1 = 512               # 512:1024   distance-1 block
J2 = 1024              # 1024:1536  distance-2 block
R3P0 = 1536            # 1536:2048  d3 P0 rhs (partner[0:256] twice)
R3P1 = 2048            # 2048:2560  d3 P1 rhs (mirror partner, full)
L4 = 2560              # 2560:3072  d4 lhsT chunks (parity-packed)
R4 = 3072              # 3072:3584  d4 rhs halves (parity-packed)
OHJ = 3584             # 3584:3588  one-hots for j1/j2 colsum rows
OH6 = 3588             # 3588:3624  one-hots for the 6 quarter-chain rows
FT_COLS = 3632

# slot ids: s0=(j0,r0) | s1..s4 = j1 r0..r3 | s5..s8 = j2 r0..r3 |
# s9,s10 = d3-P0 halves | s11,s12 = d3-P1 halves | s13,s14 = d4 halves |
# s15..s17 = (j0, r1..r3)
# outt rowsum cols: s0->0, s1..s8 -> 1..8, half-slot quarters -> 9..20
# (two cols per half-slot), j0 r1..r3 -> 21..23, CE -> 24..27.
GROUPS = [[0], [15, 1, 2], [3, 4, 5], [6, 7, 8], [13, 14, 9],
          [10, 11, 12], [16], [17]]
J0_COL = {0: 0, 1: 21, 2: 22, 3: 23}
# half-slot kind -> (first quarter's outt col, cs4 row base, lhs offsets, rhs base)
HALF_KIND = {
    "d3p0": (9, 2, (0, 128, 256, 384), R3P0),
    "d3p1": (13, 4, (256, 384, 256, 384), R3P1),
    "d4": (17, 0, (L4, L4 + 128, L4 + 256, L4 + 384), R4),
}

_CACHE = {}


def _slot_info(s):
    if s == 0:
        return ("full", 0, 0)
    if 1 <= s <= 8:
        return ("full", 1 + (s - 1) // 4, (s - 1) % 4)
    if s in (9, 10):
        return ("d3p0", None, s - 9)
    if s in (11, 12):
        return ("d3p1", None, s - 11)
    if s in (13, 14):
        return ("d4", None, s - 13)
    return ("full", 0, s - 14)


def _build_nc():
    import concourse.tile as tile
    from concourse import bacc, mybir

    f32 = mybir.dt.float32
    bf16 = mybir.dt.bfloat16
    AF = mybir.ActivationFunctionType
    OP = mybir.AluOpType

    nc = bacc.Bacc("TRN2", target_bir_lowering=False, debug=False)

    ft_d = nc.dram_tensor("ft", [D, FT_COLS], bf16, kind="ExternalInput")
    pred_d = nc.dram_tensor("pred", [128, 4 * C], f32, kind="ExternalInput")
    out_d = nc.dram_tensor("out", [128, 28], f32, kind="ExternalOutput")
    csj_d = nc.dram_tensor("csj", [2, B], f32, kind="ExternalOutput")
    cs4_d = nc.dram_tensor("cs4", [6, 256], f32, kind="ExternalOutput")

    with tile.TileContext(nc) as tc, ExitStack() as ctx:
        const = ctx.enter_context(tc.tile_pool(name="const", bufs=1))
        gpool = ctx.enter_context(tc.tile_pool(name="gp", bufs=2, space="PSUM"))
        cjpool = ctx.enter_context(tc.tile_pool(name="cj", bufs=1, space="PSUM"))
        c4pool = ctx.enter_context(tc.tile_pool(name="c4", bufs=1, space="PSUM"))
        epool = ctx.enter_context(tc.tile_pool(name="ep", bufs=4))
        small = ctx.enter_context(tc.tile_pool(name="sm", bufs=1))

        ftt = const.tile([D, FT_COLS], bf16)
        predt = const.tile([128, 4 * C], f32)
        outt = small.tile([128, 28], f32)
        scr = small.tile([128, 3 * B], bf16)
        scrp = small.tile([128, C], bf16)
        csjs = small.tile([2, B], f32)
        cs4s = small.tile([6, 256], f32)

        nc.sync.dma_start(ftt[:, 0:B], ft_d[:, 0:B])
        nc.sync.dma_start(ftt[:, B : 2 * B], ft_d[:, B : 2 * B])
        nc.sync.dma_start(ftt[:, 2 * B : 3 * B], ft_d[:, 2 * B : 3 * B])
        nc.sync.dma_start(ftt[:, 3 * B : FT_COLS], ft_d[:, 3 * B : FT_COLS])
        nc.sync.dma_start(predt[:], pred_d[:, :])

        # one shared bank: j1/j2 colsums live in partitions 0:2 until their
        # staging copy; the final group's Gram tile then overwrites it
        cshp = cjpool.tile([128, B], f32)
        cs4p = c4pool.tile([6, 256], f32)

        pg = {}
        eg = {}

        def emit_mms(g, tile_=None):
            slots = GROUPS[g]
            if tile_ is None:
                pgt = gpool.tile([128, len(slots) * B], f32, tag="pg")
            else:
                pgt = tile_
            pg[g] = pgt
            for i, s in enumerate(slots):
                kind, j, r = _slot_info(s)
                if kind == "full":
                    nc.tensor.matmul(
                        pgt[:, i * B : (i + 1) * B],
                        ftt[:, r * 128 : (r + 1) * 128],
                        ftt[:, j * B : (j + 1) * B],
                        start=True,
                        stop=True,
                    )
                else:
                    _, _, lhs_off, rhs_base = HALF_KIND[kind]
                    for h in range(2):
                        q = 2 * r + h
                        lo = lhs_off[q] if kind != "d3p0" else 128 * q
                        nc.tensor.matmul(
                            pgt[:, i * B + h * 256 : i * B + (h + 1) * 256],
                            ftt[:, lo : lo + 128],
                            ftt[:, rhs_base + (q // 2) * 256 : rhs_base + (q // 2 + 1) * 256],
                            start=True,
                            stop=True,
                        )

        def emit_exp(g, accum=None):
            egt = epool.tile([128, len(GROUPS[g]) * B], bf16, tag="eg")
            eg[g] = egt
            nc.scalar.activation(
                egt[:], pg[g][:], AF.Exp, bias=0.0, scale=10.0, accum_out=accum
            )

        def emit_rs(g):
            for i, s in enumerate(GROUPS[g]):
                kind, j, r = _slot_info(s)
                if kind == "full":
                    col = J0_COL[r] if j == 0 else s
                    nc.vector.tensor_scalar(
                        scr[:, i * B : (i + 1) * B],
                        eg[g][:, i * B : (i + 1) * B],
                        1.0, None, OP.mult, OP.add,
                        accum_out=outt[:, col : col + 1],
                    )
                else:
                    col0 = HALF_KIND[kind][0]
                    for h in range(2):
                        col = col0 + 2 * r + h
                        nc.vector.tensor_scalar(
                            scr[:, i * B + h * 256 : i * B + (h + 1) * 256],
                            eg[g][:, i * B + h * 256 : i * B + (h + 1) * 256],
                            1.0, None, OP.mult, OP.add,
                            accum_out=outt[:, col : col + 1],
                        )

        def emit_cs(g):
            for i, s in enumerate(GROUPS[g]):
                kind, j, r = _slot_info(s)
                if kind == "full":
                    if j not in (1, 2):
                        continue
                    oh = OHJ + 2 * (j - 1)
                    nc.tensor.matmul(
                        cshp[0:2, :],
                        ftt[:, oh : oh + 2],
                        eg[g][:, i * B : (i + 1) * B],
                        start=(s == 1),
                        stop=(s == 8),
                    )
                else:
                    row_base = HALF_KIND[kind][1]
                    for h in range(2):
                        q = 2 * r + h
                        row = row_base + q // 2
                        oh = OH6 + 6 * row
                        nc.tensor.matmul(
                            cs4p[:],
                            ftt[:, oh : oh + 6],
                            eg[g][:, i * B + h * 256 : i * B + (h + 1) * 256],
                            start=(s == 13 and q == 0),
                            stop=(s == 12 and q == 3),
                        )

        # Explicit pipeline schedule.  ACT order: exps 0-5, CE, 6, 7 — the
        # CE exp fills the slot where every colsum chain is closing, so
        # both staging copies + their Pool-queue DMAs hide under it and
        # only the final exp's accumulator gates the output DMA.
        emit_mms(0)
        emit_mms(1)
        emit_exp(0)
        emit_rs(0)
        emit_mms(2)
        emit_exp(1)
        emit_rs(1)
        emit_mms(3)
        emit_exp(2)
        emit_rs(2)
        emit_cs(1)
        emit_mms(4)
        emit_exp(3)
        emit_rs(3)
        emit_cs(2)
        emit_mms(5)
        emit_exp(4)
        # j1/j2 colsum chain closes with cs(3); stage + DMA in DVE slack
        emit_cs(3)
        nc.vector.tensor_copy(csjs[:], cshp[0:2, :])
        nc.gpsimd.dma_start(csj_d[:, :], csjs[:])
        emit_rs(4)
        emit_mms(6)
        emit_exp(5)
        emit_rs(5)
        # the final group's matmul reuses the retired j1/j2-colsum bank so
        # it isn't stuck waiting on the rotating pipeline banks
        emit_mms(7, tile_=cshp)
        # quarter chains (d3/d4) close with cs(5); copy + DMA before the
        # CE work occupies DVE
        emit_cs(4)
        emit_cs(5)
        nc.vector.tensor_copy(cs4s[:], cs4p[:])
        nc.gpsimd.dma_start(cs4_d[:, :], cs4s[:])
        # CE: one big exp + per-chunk DVE accums, late enough that the
        # predicts DMA long arrived, early enough to hide the cs tails
        ept = epool.tile([128, 4 * C], bf16, tag="ept")
        nc.scalar.activation(ept[:], predt[:], AF.Exp, bias=0.0, scale=1.0)
        for cchunk in range(4):
            nc.vector.tensor_scalar(
                scrp[:],
                ept[:, cchunk * C : (cchunk + 1) * C],
                1.0, None, OP.mult, OP.add,
                accum_out=outt[:, 24 + cchunk : 25 + cchunk],
            )
        emit_exp(6)
        emit_rs(6)
        # last group: single diag tile, rowsum via the exp's accumulator
        emit_exp(7, accum=outt[:, 23:24])

        nc.sync.dma_start(out_d[:, :], outt[:])

    nc.compile()
    return nc


def _get_nc():
    if "nc" not in _CACHE:
        _CACHE["nc"] = _build_nc()
    return _CACHE["nc"]


def _prep_in_maps(predicts, labels, features):
    import ml_dtypes

    feats = np.ascontiguousarray(features, dtype=np.float32)
    pred = np.ascontiguousarray(predicts, dtype=np.float32)
    f8 = feats.reshape(B, FLIP, D).transpose(1, 0, 2)  # [8,512,128], f8[c]=feats[c::8]

    ohj = np.zeros((D, 4), dtype=np.float32)
    ohj[:, 0] = 1.0   # j1 -> csj row 0
    ohj[:, 3] = 1.0   # j2 -> csj row 1
    oh6 = np.zeros((D, 36), dtype=np.float32)
    for r in range(6):
        oh6[:, 6 * r + r] = 1.0

    in_maps = []
    for a in range(FLIP):
        ft = np.zeros((D, FT_COLS), dtype=np.float32)
        ft[:, 0:B] = f8[a].T
        ft[:, J1 : J1 + B] = f8[(a + 1) % FLIP].T
        ft[:, J2 : J2 + B] = f8[(a + 2) % FLIP].T
        p3 = f8[(a + 3) % FLIP]
        ft[:, R3P0 : R3P0 + 256] = p3[0:256].T
        ft[:, R3P0 + 256 : R3P1] = p3[0:256].T
        ft[:, R3P1 : R3P1 + B] = f8[(a - 3) % FLIP].T
        p4 = f8[(a + 4) % FLIP]
        own = f8[a]
        if a < 4:
            ft[:, L4 : L4 + B] = own.T
            ft[:, R4 : R4 + 256] = p4[0:256].T
            ft[:, R4 + 256 : R4 + 512] = p4[0:256].T
        else:
            ft[:, L4 : L4 + 128] = own[256:384].T
            ft[:, L4 + 128 : L4 + 256] = own[384:512].T
            ft[:, L4 + 256 : L4 + 384] = own[256:384].T
            ft[:, L4 + 384 : R4] = own[384:512].T
            ft[:, R4 : R4 + B] = p4.T
        ft[:, OHJ:OH6] = ohj
        ft[:, OH6 : OH6 + 36] = oh6
        pb = pred[a * B : (a + 1) * B].reshape(4, 128, C).transpose(1, 0, 2)
        in_maps.append(
            {
                "ft": np.ascontiguousarray(ft).astype(ml_dtypes.bfloat16),
                "pred": np.ascontiguousarray(pb.reshape(128, 4 * C)),
            }
        )
    return in_maps


def _stitch_pair(mP0, mP1, csP0, csP1, colP0, colP1, rowP0, rowP1):
    """Assemble both rowsum directions of a split block M (P0 core holds
    cols 0:256 over all rows; P1 core holds rows 256:512 over all cols)."""
    partial = mP0[:, colP0 : colP0 + 4].T.reshape(B)      # cols 0:256, by chunk
    compl_ = np.concatenate([csP1[rowP1], csP1[rowP1 + 1]])  # cols 256:512
    s_fwd = partial + compl_
    lo = csP0[rowP0] + csP0[rowP0 + 1]                    # mirror rows 0:256
    hi = np.empty(256)
    hi[0:128] = mP1[:, colP1] + mP1[:, colP1 + 2]         # rows 256:384
    hi[128:256] = mP1[:, colP1 + 1] + mP1[:, colP1 + 3]   # rows 384:512
    s_rev = np.concatenate([lo, hi])
    return s_fwd, s_rev


def _combine(outs, predicts, labels, features):
    """Host-side O(N*D) combine: reroute per-block sums between the
    ordered halves and apply the closed-form first-order series."""
    import ml_dtypes

    feats = np.asarray(features, dtype=np.float32)
    f8 = feats.reshape(B, FLIP, D).transpose(1, 0, 2).astype(np.float64)
    fb8 = f8.astype(ml_dtypes.bfloat16).astype(np.float64)  # device-side values

    dv = np.einsum("apd,bpd->abp", f8, f8)

    m = {}
    csj = {}
    cs4 = {}
    for c in range(FLIP):
        m[c] = np.asarray(outs[c]["out"], np.float64)
        csj[c] = np.asarray(outs[c]["csj"], np.float64)
        cs4[c] = np.asarray(outs[c]["cs4"], np.float64)

    S1 = {}
    for c in range(FLIP):
        for j in range(3):
            b = (c + j) % FLIP
            cols = [J0_COL[r] for r in range(4)] if j == 0 else [1 + 4 * (j - 1) + r for r in range(4)]
            S1[(c, b)] = m[c][:, cols].T.reshape(B)
        for j in (1, 2):
            S1[((c + j) % FLIP, c)] = csj[c][j - 1]

    for b in range(FLIP):  # distance-3 pairs, P0 = core b, P1 = core b+3
        bp = (b + 3) % FLIP
        s_fwd, s_rev = _stitch_pair(m[b], m[bp], cs4[b], cs4[bp], 9, 13, 2, 4)
        S1[(b, bp)] = s_fwd
        S1[(bp, b)] = s_rev
    for b in range(4):     # distance-4 pairs, P0 = core b, P1 = core b+4
        bp = b + 4
        s_fwd, s_rev = _stitch_pair(m[b], m[bp], cs4[b], cs4[bp], 17, 17, 0, 0)
        S1[(b, bp)] = s_fwd
        S1[(bp, b)] = s_rev

    # remove the raw diagonal exp from the own-block rowsums.  chunks r0-r2
    # were summed from bf16 E by DVE; chunk r3 rides the ACT accumulator
    # (f32 activation results), so skip the bf16 rounding there.
    S10 = {}
    for c in range(FLIP):
        gpp = np.einsum("pd,pd->p", fb8[c], fb8[c])
        dg = np.exp(10.0 * gpp).astype(np.float32)
        dgb = dg.astype(ml_dtypes.bfloat16).astype(np.float64)
        dgb[384:512] = dg[384:512]
        S10[c] = S1[(c, c)] - dgb

    nce = 0.0
    for a in range(FLIP):
        for b in range(FLIP):
            d = dv[a, b]
            if a == b:
                N1 = 2.0 * S10[a]
                Dv = N1 + E10
                half = 10.0 * d - np.log(Dv) - N1 / Dv
                nce += 2.0 * half.sum()
            else:
                N1 = S10[a] + S1[(a, b)]
                half = (
                    10.0 * d
                    - np.log(N1)
                    - 1.0
                    - np.log1p(-np.exp(10.0 * d) / N1)
                )
                nce += half.sum()

    # CE: device exp-sums + host label gather
    pred = np.asarray(predicts, dtype=np.float64)
    lab = np.asarray(labels).astype(np.int64)
    xl = pred[np.arange(N), lab]
    ce = -xl.sum()
    for c in range(FLIP):
        se = m[c][:, 24:28]  # se[p, cc] = sum_k exp(pred[c*512+cc*128+p, k])
        ce += np.log(se).T.reshape(B).sum()

    val = ALPHA * (-(nce) / 1024.0) + ce / N
    return np.array(val, dtype=np.float32)


def _run_hw(in_maps, trace=False):
    from concourse.bass_utils import run_bass_kernel_spmd

    nc = _get_nc()
    return run_bass_kernel_spmd(nc, in_maps, core_ids=list(range(FLIP)), trace=trace)


def kernel(predicts, labels, features, indexs=None, **_):
    in_maps = _prep_in_maps(predicts, labels, features)
    res = _run_hw(in_maps)
    return _combine(res.results, predicts, labels, features)


def kernel_sim(predicts, labels, features, indexs=None, **_):
    """CoreSim (CPU simulator) path for fast correctness iteration."""
    from concourse.bass_interp import CoreSim

    nc = _get_nc()
    in_maps = _prep_in_maps(predicts, labels, features)
    outs = []
    for a in range(FLIP):
        sim = CoreSim(nc, trace=False)
        for k, v in in_maps[a].items():
            sim.tensor(k)[:] = v
        sim.simulate()
        outs.append({k: np.array(sim.tensor(k)) for k in ("out", "csj", "cs4")})
    return _combine(outs, predicts, labels, features)
